# revision 1
# baseline (speedup 1.0000x reference)
"""Trainium2 Bass kernel: 1-layer transformer block w/ ALiBi bidirectional attention.

Sharding: data-parallel over batch (B=8) across 8 NeuronCores; zero collectives.

v2 (bf16): all matmuls run in bf16 (1 cyc/row on PE vs 4 for fp32, and
LDWEIGHTS gets fast-weight-load). Residual stream / LN stats / softmax
normalization stay fp32. Other changes vs v1:
  - x is transposed + cast to bf16 on HOST; no on-device xT transposes.
  - q/k projections emit per-head [64, S] psum chunks directly into the
    augmented [65, S] qTa/kTa tiles -> no SBUF->SBUF head-split DMAs.
  - probs@V computed s-major: out[s, 65] = expT_chunk.T @ v_aug, so the
    softmax denominator lands in column 64 and normalization is a
    per-partition scalar multiply; no per-head transposes.
  - ALiBi: per-s term rides the augmented q row (bf16 rounding of it is a
    per-s additive exponent error that cancels exactly in softmax);
    per-t term is the fp32 per-partition ACT bias of the fused exp.
  - LN scale/bias folded into following weight matrices host-side.
"""

import sys

import ml_dtypes
import numpy as np

sys.path.insert(0, "/opt/trn_rl_repo")

import concourse.bass as bass  # noqa: E402
from concourse import bacc  # noqa: E402
import concourse.tile as tile  # noqa: E402
from concourse import mybir  # noqa: E402
from concourse.bass_utils import run_bass_kernel_spmd  # noqa: E402

F32 = mybir.dt.float32
BF = mybir.dt.bfloat16
AF = mybir.ActivationFunctionType
OP = mybir.AluOpType

P = 128
B = 8
S = 1024
D = 512
H = 8
HD = 64
FFN = 4 * D
SM = S // P  # 8 sequence chunks
DK = D // P  # 4 feature chunks
FK = FFN // P  # 16 ffn chunks
EPS = 1e-5
N_CORES = 8

BF_NP = ml_dtypes.bfloat16


def _slopes():
    half = H // 2
    base = 24.0 ** (1.0 / half)
    return (1.0 / base ** np.arange(1, half + 1)).astype(np.float64)


def _fwd(h):
    return h < H // 2


# per (head, j) score-tile geometry for the transposed scores [t=j*128+p, s]
def _s_range(h, j):
    if _fwd(h):  # keep t <= s : s-chunks j..7
        return j * P, S - j * P
    else:  # keep t >= s : s-chunks 0..j
        return 0, (j + 1) * P


def _eoff(h, j):
    off = 0
    for jj in range(j):
        off += _s_range(h, jj)[1]
    return off


def _ewidth(h):
    return _eoff(h, SM - 1) + _s_range(h, SM - 1)[1]  # = 4608


def build_nc(gelu_mode="gelu"):
    nc = bacc.Bacc("TRN2", target_bir_lowering=False, debug=False)

    def din(name, shape, dt=F32):
        return nc.dram_tensor(name, list(shape), dt, kind="ExternalInput").ap()

    d = {}
    d["xT"] = din("xT", (D, S), BF)
    d["w_in"] = din("w_in", (D, D), BF)
    d["wq"] = din("wq", (D, D), BF)
    d["wk"] = din("wk", (D, D), BF)
    d["wv"] = din("wv", (D, D), BF)
    d["wo"] = din("wo", (D, D), BF)
    d["bo"] = din("bo", (D,))
    d["w1"] = din("w1", (D, FFN), BF)
    d["w2"] = din("w2", (FFN, D), BF)
    d["b2"] = din("b2", (D,))
    d["w_out"] = din("w_out", (D, D), BF)
    d["bqc"] = din("bqc", (HD, H))
    d["b1c"] = din("b1c", (P, FK))
    d["b_in"] = din("b_in", (D,))
    d["bv"] = din("bv", (D,))
    d["b_out"] = din("b_out", (D,))
    d["qrow"] = din("qrow", (H, S), BF)
    d["tb"] = din("tb", (P, H * SM))
    d["maskf"] = din("maskf", (P, P), BF)
    d["maskb"] = din("maskb", (P, P), BF)
    d["ident"] = din("ident", (P, P), BF)
    d["out"] = nc.dram_tensor("out", [S, D], F32, kind="ExternalOutput").ap()

    with tile.TileContext(nc) as tc:
        _emit(nc, tc, d, gelu_mode)
    nc.compile()
    return nc


def _emit(nc, tc, d, gelu_mode):
    pool = tc.alloc_tile_pool

    pc = pool(name="consts", bufs=1)
    pw = pool(name="weights", bufs=1)  # all weights resident, bf16
    ph = pool(name="resid", bufs=2)  # tag "h": h1, h2, h3 rotate (fp32)
    phT = pool(name="transposed", bufs=2)  # tag "hT": hn1T,attnT2,hn2T,hn3T
    psm = pool(name="smalls", bufs=4)
    phn = pool(name="hn_nat", bufs=1)
    pg = pool(name="gelu", bufs=2)
    posb = pool(name="outsb", bufs=2)
    pattn = pool(name="attn_nat", bufs=1)
    pva = pool(name="vaug", bufs=1)
    pqk = pool(name="qkheads", bufs=3)
    pexp = pool(name="expT", bufs=2)

    ps_mm = pool(name="ps_mm", bufs=2, space="PSUM")
    # 4-deep rotation: score matmuls run ahead of ACT exp; doubles as the
    # 4 live FFN2 accumulators
    ps_acc = pool(name="ps_acc", bufs=4, space="PSUM")
    ps_tr = pool(name="ps_tr", bufs=2, space="PSUM")

    # ---- weights (bf16), staged early; all fit resident ----
    def wload(name, shape, view):
        t = pw.tile(shape, BF, tag=name)
        nc.sync.dma_start(out=t, in_=view)
        return t

    # each dma_start costs ~1us of issue time on its queue engine, so the
    # startup-critical tensors (ident for warm-up, w_in, x) go first as
    # single descriptors
    identB = pc.tile([P, P], BF, tag="ident")
    nc.sync.dma_start(out=identB, in_=d["ident"])
    win_sb = wload("w_in", [P, DK, D], d["w_in"].rearrange("(c p) n -> p c n", p=P))
    xT_sb = wload("xT", [P, DK, S], d["xT"].rearrange("(c p) n -> p c n", p=P))
    wq_sb = wload("wq", [P, DK, D], d["wq"].rearrange("(c p) n -> p c n", p=P))
    wk_sb = wload("wk", [P, DK, D], d["wk"].rearrange("(c p) n -> p c n", p=P))
    wv_sb = wload("wv", [P, DK, D], d["wv"].rearrange("(c p) n -> p c n", p=P))
    wo_sb = wload("wo", [P, DK, D], d["wo"].rearrange("(c p) n -> p c n", p=P))
    w1_sb = wload("w1", [P, DK, FFN], d["w1"].rearrange("(c p) n -> p c n", p=P))
    w2_sb = wload("w2", [P, FK, D], d["w2"].rearrange("(c p) n -> p c n", p=P))
    wout_sb = wload("w_out", [P, DK, D], d["w_out"].rearrange("(c p) n -> p c n", p=P))

    # ---- constants ----
    maskf = pc.tile([P, P], BF, tag="maskf")
    nc.sync.dma_start(out=maskf, in_=d["maskf"])
    maskb = pc.tile([P, P], BF, tag="maskb")
    nc.sync.dma_start(out=maskb, in_=d["maskb"])
    tb = pc.tile([P, H * SM], F32, tag="tb")
    nc.sync.dma_start(out=tb, in_=d["tb"])
    bqc = pc.tile([HD, H], F32, tag="bqc")
    nc.sync.dma_start(out=bqc, in_=d["bqc"])
    b1c = pc.tile([P, FK], F32, tag="b1c")
    nc.sync.dma_start(out=b1c, in_=d["b1c"])
    b1cs = pc.tile([P, FK], F32, tag="b1cs")
    nc.any.tensor_scalar(b1cs, b1c, scalar1=1.702, scalar2=None, op0=OP.mult)

    def bcast(name, shape=None):
        t = pc.tile(shape or [P, D], F32, tag=name + "B")
        nc.gpsimd.dma_start(out=t, in_=d[name].partition_broadcast(P))
        return t

    epsc = pc.tile([P, 1], F32, tag="epsc")
    nc.any.memset(epsc, EPS)

    boB = bcast("bo")
    b2B = bcast("b2")
    binB = bcast("b_in")
    bvB = bcast("bv", [P, H, HD])
    boutB = bcast("b_out")

    # PE clock keep-warm: the HAM gate drops the PE to 1.2GHz after ~3.4us
    # of idleness and takes another ~3.4us of activity to lift it back.
    # Dependency-free dummy transposes (53ns each) fill known stall windows
    # so the matmuls that follow run at full clock.
    def warm(n):
        for _ in range(n):
            wt = ps_tr.tile([P, P], BF, tag="tr", name="warm")
            nc.tensor.transpose(wt, identB, identB)

    warm(48)  # initial DMA wait

    # ---- h1 = x @ w_in + b_in  (natural fp32, residual base) ----
    h1 = ph.tile([P, SM, D], F32, tag="h")

    def emit_h1(m):
        ps = ps_mm.tile([P, D], F32, tag="mm")
        for dk in range(DK):
            nc.tensor.matmul(
                ps,
                xT_sb[:, dk, m * P : (m + 1) * P],
                win_sb[:, dk, :],
                start=(dk == 0),
                stop=(dk == DK - 1),
            )
        nc.vector.tensor_tensor(out=h1[:, m, :], in0=ps, in1=binB, op=OP.add)

    def ln_rows(srcs):
        # LayerNorm for several rows, issued step-batched so the DVE queue
        # never parks behind a cross-engine (ACT sqrt) round-trip.
        # scale/bias are folded into the following weights host-side.
        n = len(srcs)
        st, mv, sq, rs, ng, hns = {}, {}, {}, {}, {}, {}
        for i, src in enumerate(srcs):
            st[i] = psm.tile([P, 6], F32, tag=f"st{i}", name=f"st{i}")
            nc.vector.bn_stats(st[i], src)
        for i in range(n):
            mv[i] = psm.tile([P, 2], F32, tag=f"mv{i}", name=f"mv{i}")
            nc.vector.bn_aggr(mv[i], st[i])
        for i in range(n):
            sq[i] = psm.tile([P, 1], F32, tag=f"sq{i}", name=f"sq{i}")
            nc.scalar.activation(sq[i], mv[i][:, 1:2], AF.Sqrt, bias=epsc)
        for i in range(n):
            rs[i] = psm.tile([P, 1], F32, tag=f"rs{i}", name=f"rs{i}")
            nc.vector.reciprocal(rs[i], sq[i])
        for i in range(n):
            ng[i] = psm.tile([P, 1], F32, tag=f"ng{i}", name=f"ng{i}")
            nc.vector.tensor_scalar(
                ng[i], mv[i][:, 0:1], scalar1=rs[i], scalar2=-1.0,
                op0=OP.mult, op1=OP.mult,
            )
        for i, src in enumerate(srcs):
            hns[i] = phn.tile([P, D], BF, tag=f"hn{i}", name=f"hn{i}")
            nc.scalar.activation(hns[i], src, AF.Identity, bias=ng[i], scale=rs[i])
        return [hns[i] for i in range(n)]

    def ln_chunk(src):
        return ln_rows([src])[0]

    def transpose_row(hT, m, src):
        # transpose the 4 [128,128] blocks of src into one psum tile, then
        # write hT[:, :, m*P:(m+1)*P] with a single strided DVE copy
        t4 = ps_tr.tile([P, DK, P], BF, tag="tr")
        for dk in range(DK):
            nc.tensor.transpose(
                t4[:, dk, :], src[:, dk * P : (dk + 1) * P], identB
            )
        nc.vector.tensor_copy(hT[:, :, m * P : (m + 1) * P], t4)

    # hn1T = LN1(h1) transposed [d, s] bf16, and the v projection, both
    # emitted row-by-row staggered behind h1 so the PE stream never parks
    # behind a serial LN chain
    hn1T = phT.tile([P, DK, S], BF, tag="hT")
    v_aug = pva.tile([P, SM, H, HD + 1], BF, tag="vaug")

    def emit_v(m):
        psv = ps_mm.tile([P, H, HD], F32, tag="mm", name="psv")
        for dk in range(DK):
            nc.tensor.matmul(
                psv,
                hn1T[:, dk, m * P : (m + 1) * P],
                wv_sb[:, dk, :],
                start=(dk == 0),
                stop=(dk == DK - 1),
            )
        nc.vector.tensor_tensor(out=v_aug[:, m, :, 0:HD], in0=psv, in1=bvB, op=OP.add)
        nc.gpsimd.memset(v_aug[:, m, :, HD : HD + 1], 1.0)

    for m in range(SM):
        emit_h1(m)
    hn1s = ln_rows([h1[:, m, :] for m in range(SM)])
    for m in range(SM):
        transpose_row(hn1T, m, hn1s[m])
        emit_v(m)

    # ---- attention: software-pipelined so PE never waits on ACT exp ----
    # issue order per step: qk GEMMs(h), scores(h-1), probs@V(h-2); the exp
    # of head h-1 runs on ACT while PE does head h's projections.
    attn_nat = pattn.tile([P, SM, D], BF, tag="attn")
    qk_t = {}
    exp_t = {}

    def emit_qk(h):
        qTa = pqk.tile([HD + 1, S], BF, tag="qTa", name=f"qTa{h}")
        nc.gpsimd.dma_start(out=qTa[HD : HD + 1, :], in_=d["qrow"][h : h + 1, :])
        kTa = pqk.tile([HD + 1, S], BF, tag="kTa", name=f"kTa{h}")
        nc.gpsimd.memset(kTa[HD : HD + 1, :], 1.0)
        for w_sb, dst, is_q in ((wq_sb, qTa, True), (wk_sb, kTa, False)):
            for half in range(2):
                psq = ps_mm.tile([HD, D], F32, tag="mm", name="psq")
                for dk in range(DK):
                    nc.tensor.matmul(
                        psq,
                        w_sb[:, dk, h * HD : (h + 1) * HD],
                        hn1T[:, dk, half * 512 : (half + 1) * 512],
                        start=(dk == 0),
                        stop=(dk == DK - 1),
                    )
                if is_q:
                    nc.vector.tensor_scalar(
                        dst[0:HD, half * 512 : (half + 1) * 512],
                        psq,
                        scalar1=bqc[:, h : h + 1],
                        scalar2=None,
                        op0=OP.add,
                    )
                else:
                    # k bias dropped: it only shifts scores by a per-s
                    # constant, which softmax normalization cancels exactly
                    nc.vector.tensor_copy(
                        dst[0:HD, half * 512 : (half + 1) * 512], psq
                    )
        qk_t[h] = (qTa, kTa)

    def emit_scores(h):
        qTa, kTa = qk_t[h]
        expT = pexp.tile([P, _ewidth(h)], BF, tag="expT", name=f"expT{h}")
        for j in range(SM):
            s0, w = _s_range(h, j)
            eo = _eoff(h, j)
            off = 0
            while off < w:
                pw_ = min(512, w - off)
                pss = ps_acc.tile([P, pw_], F32, tag="acc", name="pss")
                nc.tensor.matmul(
                    pss,
                    kTa[:, j * P : (j + 1) * P],
                    qTa[:, s0 + off : s0 + off + pw_],
                    start=True,
                    stop=True,
                )
                nc.scalar.activation(
                    expT[:, eo + off : eo + off + pw_],
                    pss,
                    AF.Exp,
                    bias=tb[:, h * SM + j : h * SM + j + 1],
                    scale=0.125,
                )
                off += pw_
            # mask the diagonal 128x128 block (keep t<=s fwd / t>=s bwd)
            dg = eo if _fwd(h) else eo + j * P
            msk = maskf if _fwd(h) else maskb
            nc.gpsimd.tensor_tensor(
                out=expT[:, dg : dg + P],
                in0=expT[:, dg : dg + P],
                in1=msk,
                op=OP.mult,
            )
        exp_t[h] = expT

    def emit_pv(h):
        # probs @ V, s-major: out[s, 65]; col 64 = softmax denominator
        expT = exp_t.pop(h)
        qk_t.pop(h)
        for m in range(SM):
            js = list(range(0, m + 1)) if _fwd(h) else list(range(m, SM))
            pv = ps_mm.tile([P, HD + 1], F32, tag="mm", name="pvps")
            for i, j in enumerate(js):
                s0, _w = _s_range(h, j)
                col = _eoff(h, j) + (m * P - s0)
                nc.tensor.matmul(
                    pv,
                    expT[:, col : col + P],
                    v_aug[:, j, h, :],
                    start=(i == 0),
                    stop=(i == len(js) - 1),
                )
            rinv = psm.tile([P, 1], F32, tag="rinv")
            nc.vector.reciprocal(rinv, pv[:, HD : HD + 1])
            # alternate DVE/ACT so the norm queue drains twice as fast at
            # the attention -> wo boundary
            if h % 2 == 0:
                nc.vector.tensor_scalar(
                    attn_nat[:, m, h * HD : (h + 1) * HD],
                    pv[:, 0:HD],
                    scalar1=rinv,
                    scalar2=None,
                    op0=OP.mult,
                )
            else:
                nc.scalar.activation(
                    attn_nat[:, m, h * HD : (h + 1) * HD],
                    pv[:, 0:HD],
                    AF.Copy,
                    scale=rinv,
                )

    for step in range(H + 2):
        if step < H:
            emit_qk(step)
        if 0 <= step - 1 < H:
            emit_scores(step - 1)
        if 0 <= step - 2 < H:
            emit_pv(step - 2)

    # ---- attnT2 + wo + h2 + LN2, per s-chunk; LN2 rows 4..7 deferred into
    # the FFN half-0 matmul stream so PE is never parked behind the LN chain
    attnT2 = phT.tile([P, DK, S], BF, tag="hT")
    h2 = ph.tile([P, SM, D], F32, tag="h")
    hn2T = phT.tile([P, DK, S], BF, tag="hT")

    def emit_wo_h2(m):
        transpose_row(attnT2, m, attn_nat[:, m, :])
        ps = ps_mm.tile([P, D], F32, tag="mm", name="pswo")
        for dk in range(DK):
            nc.tensor.matmul(
                ps,
                attnT2[:, dk, m * P : (m + 1) * P],
                wo_sb[:, dk, :],
                start=(dk == 0),
                stop=(dk == DK - 1),
            )
        nc.vector.tensor_tensor(out=h2[:, m, :], in0=ps, in1=h1[:, m, :], op=OP.add)
        nc.gpsimd.tensor_tensor(out=h2[:, m, :], in0=h2[:, m, :], in1=boB, op=OP.add)

    for m in range(SM):
        emit_wo_h2(m)
    hn2s = ln_rows([h2[:, m, :] for m in range(4)])
    for m in range(4):
        transpose_row(hn2T, m, hn2s[m])

    # ---- FFN: h3 = h2 + gelu(hn2 @ w1 + b1) @ w2 + b2 ----
    h3 = ph.tile([P, SM, D], F32, tag="h")
    hn3T = phT.tile([P, DK, S], BF, tag="hT")
    out_view = d["out"].rearrange("(c p) n -> p c n", p=P)

    def emit_wout(m):
        ps = ps_mm.tile([P, D], F32, tag="mm", name="psout")
        for dk in range(DK):
            nc.tensor.matmul(
                ps,
                hn3T[:, dk, m * P : (m + 1) * P],
                wout_sb[:, dk, :],
                start=(dk == 0),
                stop=(dk == DK - 1),
            )
        osb = posb.tile([P, D], F32, tag="osb")
        nc.vector.tensor_tensor(out=osb, in0=ps, in1=boutB, op=OP.add)
        nc.sync.dma_start(out=out_view[:, m, :], in_=osb)

    def emit_out_tail(m):
        # LNf row + output projection + store, interleaved into FFN half-1
        hn = ln_chunk(h3[:, m, :])
        transpose_row(hn3T, m, hn)
        emit_wout(m)

    def emit_h3(m, acc):
        nc.vector.tensor_tensor(out=h3[:, m, :], in0=acc, in1=h2[:, m, :], op=OP.add)
        nc.gpsimd.tensor_tensor(out=h3[:, m, :], in0=h3[:, m, :], in1=b2B, op=OP.add)

    for half in range(2):
        accs = []
        for mm in range(4):
            accs.append(ps_acc.tile([P, D], F32, tag="acc", name=f"ff2ps{mm}"))
        for kc in range(FK):
            ps1 = ps_mm.tile([P, 512], F32, tag="mm", name="ff1ps")
            for dk in range(DK):
                nc.tensor.matmul(
                    ps1,
                    w1_sb[:, dk, kc * P : (kc + 1) * P],
                    hn2T[:, dk, half * 512 : (half + 1) * 512],
                    start=(dk == 0),
                    stop=(dk == DK - 1),
                )
            gt = pg.tile([P, 512], BF, tag="gt")
            if gelu_mode == "gelu":
                nc.scalar.activation(gt, ps1, AF.Gelu, bias=b1c[:, kc : kc + 1])
            else:  # CoreSim lacks Gelu: x*sigmoid(1.702x) stand-in
                sg = pg.tile([P, 512], F32, tag="sg")
                nc.scalar.activation(
                    sg, ps1, AF.Sigmoid, bias=b1cs[:, kc : kc + 1], scale=1.702
                )
                xb = pg.tile([P, 512], F32, tag="xb")
                nc.any.tensor_scalar(
                    xb, ps1, scalar1=b1c[:, kc : kc + 1], scalar2=None, op0=OP.add
                )
                nc.any.tensor_tensor(out=gt, in0=sg, in1=xb, op=OP.mult)
            for mm in range(4):
                nc.tensor.matmul(
                    accs[mm],
                    gt[:, mm * P : (mm + 1) * P],
                    w2_sb[:, kc, :],
                    start=(kc == 0),
                    stop=(kc == FK - 1),
                )
            # interleave deferred LN rows / output tails into the MM stream
            if half == 0:
                if kc % 4 == 2:
                    hn = ln_chunk(h2[:, 4 + kc // 4, :])
                    transpose_row(hn2T, 4 + kc // 4, hn)
            elif kc in (5, 9, 13):
                emit_out_tail((kc - 5) // 4)
        for mm in range(4):
            emit_h3(half * 4 + mm, accs[mm])
        if half == 1:
            emit_out_tail(3)

    # final 4 tails: step-batched LNf, then the projections
    hn3s = ln_rows([h3[:, m, :] for m in range(4, SM)])
    for i, m in enumerate(range(4, SM)):
        transpose_row(hn3T, m, hn3s[i])
        emit_wout(m)

    for p_ in (ps_tr, ps_acc, ps_mm, pexp, pqk, pva, pattn, posb, pg, phn,
               psm, phT, ph, pw, pc):
        p_.release()


def host_prep(inputs):
    """Fold LN affine params into weights; build ALiBi helper tensors."""
    f = lambda k: np.asarray(inputs[k], dtype=np.float64)
    ln1_s, ln1_b = f("ln1_s"), f("ln1_b")
    ln2_s, ln2_b = f("ln2_s"), f("ln2_b")
    lnf_s, lnf_b = f("lnf_s"), f("lnf_b")
    wq, bq = f("wq"), f("bq")
    wk = f("wk")
    wv, bv = f("wv"), f("bv")
    w1, b1 = f("w1"), f("b1")
    w_out, b_out = f("w_out"), f("b_out")

    wq_f = ln1_s[:, None] * wq
    bq_f = (bq + ln1_b @ wq).astype(np.float32)
    wk_f = ln1_s[:, None] * wk
    wv_f = ln1_s[:, None] * wv
    bv_f = (bv + ln1_b @ wv).astype(np.float32)
    w1_f = ln2_s[:, None] * w1
    b1_f = (b1 + ln2_b @ w1).astype(np.float32)
    wout_f = lnf_s[:, None] * w_out
    bout_f = (b_out + lnf_b @ w_out).astype(np.float32)

    sl = _slopes()
    qrow = np.zeros((H, S), np.float32)
    tb = np.zeros((P, H * SM), np.float32)
    s_idx = np.arange(S, dtype=np.float64)
    p_idx = np.arange(P, dtype=np.float64)
    for h in range(H):
        sgn = -1.0 if h < H // 2 else 1.0  # sign of the per-s row term
        qrow[h] = (sgn * 8.0 * sl[h % 4] * s_idx).astype(np.float32)
        for j in range(SM):
            tb[:, h * SM + j] = (-sgn * sl[h % 4] * (j * P + p_idx)).astype(
                np.float32
            )
    maskf = np.triu(np.ones((P, P), np.float32))  # keep t <= s (p <= c)
    maskb = np.tril(np.ones((P, P), np.float32))  # keep t >= s (p >= c)

    bf = lambda a: np.ascontiguousarray(np.asarray(a, np.float32).astype(BF_NP))
    common = {
        "w_in": bf(inputs["w_in"]),
        "b_in": np.asarray(inputs["b_in"], np.float32),
        "wq": bf(wq_f),
        "wk": bf(wk_f),
        "wv": bf(wv_f),
        "wo": bf(inputs["wo"]),
        "bo": np.asarray(inputs["bo"], np.float32),
        "w1": bf(w1_f),
        "w2": bf(inputs["w2"]),
        "b2": np.asarray(inputs["b2"], np.float32),
        "w_out": bf(wout_f),
        "b_out": bout_f,
        "bqc": np.ascontiguousarray(bq_f.reshape(H, HD).T),
        "b1c": np.ascontiguousarray(b1_f.reshape(FK, P).T),
        "bv": bv_f,
        "qrow": bf(qrow),
        "tb": tb,
        "maskf": bf(maskf),
        "maskb": bf(maskb),
        "ident": bf(np.eye(P, dtype=np.float32)),
    }
    return common


def core_map(common, x, i):
    xT = np.ascontiguousarray(np.asarray(x[i], np.float32).T.astype(BF_NP))
    return dict(common, xT=xT)


_NC_CACHE = {}


def get_nc(gelu_mode="gelu"):
    if gelu_mode not in _NC_CACHE:
        _NC_CACHE[gelu_mode] = build_nc(gelu_mode)
    return _NC_CACHE[gelu_mode]


def run(inputs, trace=False, tmpdir=None):
    common = host_prep(inputs)
    x = np.asarray(inputs["x"], np.float32)
    in_maps = [core_map(common, x, i) for i in range(N_CORES)]
    nc = get_nc()
    res = run_bass_kernel_spmd(
        nc, in_maps, core_ids=list(range(N_CORES)), trace=trace, tmpdir=tmpdir
    )
    out = np.stack([res.results[i]["out"] for i in range(N_CORES)])
    return out.astype(np.float32), res


def kernel(**inputs):
    out, _ = run(inputs, trace=False)
    return out



# revision 30
# speedup vs baseline: 1.2823x; 1.2823x over previous
"""Trainium2 Bass kernel: 1-layer transformer block w/ ALiBi bidirectional attention.

Sharding: data-parallel over batch (B=8) across 8 NeuronCores; zero collectives.

v3 (banded attention + pipeline rework), on top of v2's bf16 scheme:
  - Banded attention: ALiBi slopes decay so fast that blocks beyond
    B={2,2,3,5} (per slope) 128-blocks from the diagonal underflow to
    exactly 0 after exp (dropped mass < e^-21); scores/exp/pv do ~56%
    of the full masked work.  Validated ≤2e-12 vs full softmax.
  - q/k projections paired: full 128-wide stationary (2 heads per
    matmul) halves qk PE time; psum evacuated by ONE full-width DVE
    copy into a scratch tile, then two SBUF->SBUF DMAs partition-shift
    the per-head [64,S] halves into the augmented qTa/kTa tiles.
  - ALiBi per-t term rides the score matmul as a hi/lo pair of bf16
    aug rows (exact to 2^-18), so exp needs NO per-chunk bias and one
    ACT exp op spans a [128,1024] 2-bank psum tile: far fewer ops on
    the 352-cycle-overhead ACT engine.
  - LN rsqrt = exp(-0.5*ln(var+eps)): both Ln and Exp live in the
    natural_log_exp_and_others ACT table set (the insertion pass is
    fed a filtered table list so it picks that set), so the only table
    swaps are ln_exp -> gelu -> ln_exp (3 loads vs 9).
  - FFN2 runs per-s-chunk after each half's FFN1 (gt staged in SBUF),
    needing 1 psum accumulator instead of 4; psum plan is exactly 8
    banks: 2x[128,512] + 2x[128,1024] + 2x aux.
  - All-zero bias tensors (this model's setup) detected at runtime ->
    bias adds drop to plain copies/TTs.
  - HAM warm-up uses real matmuls (transposes don't count as PE-busy).
"""

import sys
import types

import ml_dtypes
import numpy as np

sys.path.insert(0, "/opt/trn_rl_repo")

import concourse.bass as bass  # noqa: E402
from concourse import bacc  # noqa: E402
import concourse.tile as tile  # noqa: E402
from concourse import mybir  # noqa: E402
from concourse.bass_utils import run_bass_kernel_spmd  # noqa: E402
import bass_rust as _bass_rust  # noqa: E402
from concourse.hw_specs import get_activation_tables  # noqa: E402

F32 = mybir.dt.float32
BF = mybir.dt.bfloat16
AF = mybir.ActivationFunctionType
OP = mybir.AluOpType

P = 128
B = 8
S = 1024
D = 512
H = 8
HD = 64
FFN = 4 * D
SM = S // P  # 8 sequence chunks
DK = D // P  # 4 feature chunks
FK = FFN // P  # 16 ffn chunks
EPS = 1e-5
N_CORES = 8
AUG = 3  # aug rows: (qrow|ones), (ones|krow_hi), (ones|krow_lo)

BF_NP = ml_dtypes.bfloat16

BANDS = [2, 2, 3, 5]  # kept 128-blocks (incl. diagonal) per slope index


def _slopes():
    half = H // 2
    base = 24.0 ** (1.0 / half)
    return (1.0 / base ** np.arange(1, half + 1)).astype(np.float64)


def _fwd(h):
    return h < H // 2


def _band(h):
    return BANDS[h % 4]


def _group(h, j):
    # s-range of score block-column group for t-chunk j (dense band)
    Bh = _band(h)
    if _fwd(h):  # keep t <= s, s - t < Bh*128
        s0 = j * P
        s1 = min(S, (j + Bh) * P)
    else:  # keep t >= s, t - s < Bh*128
        s0 = max(0, (j - Bh + 1) * P)
        s1 = (j + 1) * P
    return s0, s1 - s0


def _eoffs(h):
    offs, off = [], 0
    for j in range(SM):
        offs.append(off)
        off += _group(h, j)[1]
    return offs, off


def _score_tiles(h):
    """Chunk head h's dense score layout into psum tiles of <=1024 cols.

    Returns [(tile_off, tile_w, [(j, qTa_src_col, dst_col_in_tile, w)...])].
    Matmul pieces never cross an absolute 512 (bank) boundary.
    """
    offs, ew = _eoffs(h)
    pieces = []
    for j in range(SM):
        s0, w = _group(h, j)
        off = offs[j]
        pos = 0
        while pos < w:
            lim = 512 - ((off + pos) % 512)
            pw = min(w - pos, lim)
            pieces.append((j, s0 + pos, off + pos, pw))
            pos += pw
    tiles = []
    for t0 in range(0, ew, 1024):
        tw = min(1024, ew - t0)
        tp = [(j, ss, do - t0, w) for (j, ss, do, w) in pieces if t0 <= do < t0 + tw]
        tiles.append((t0, tw, tp))
    return tiles


def _diag_off(h, j):
    offs, _ = _eoffs(h)
    s0, _w = _group(h, j)
    return offs[j] + (j * P - s0)


def _pv_js(h, m):
    Bh = _band(h)
    if _fwd(h):
        return list(range(max(0, m - Bh + 1), m + 1))
    return list(range(m, min(SM, m + Bh)))


def _pv_col(h, j, m):
    offs, _ = _eoffs(h)
    s0, _w = _group(h, j)
    return offs[j] + (m * P - s0)


def _ew(h):
    return _eoffs(h)[1]


EW_S = _ew(0)  # 1920 (band 2 heads)
EW_L = _ew(3)  # 3840 (band 5; band-3 heads fit too)


def _patched_insert_act_table_loads(self):
    """Feed the table-load pass a filtered set list so Exp and Ln both
    resolve to natural_log_exp_and_others (greedy first-match would
    otherwise alternate exp_and_others / natural_log per call)."""
    has_activation = any(
        isinstance(i, mybir.InstActivation)
        for b in self.main_func.blocks
        for i in b.instructions
    )
    if not has_activation:
        return
    tables = []
    for name, fns in get_activation_tables(self.m.arch).items():
        fns = set(fns)
        if name in ("exp_and_others", "exp_and_friends"):
            fns.discard(AF.Exp)
        if name == "natural_log":
            fns.discard(AF.Ln)
        tables.append((name, fns))
    _bass_rust.insert_act_table_loads(self, tables)


def build_nc(gelu_mode="gelu", zero_bias=True):
    nc = bacc.Bacc("TRN2", target_bir_lowering=False, debug=False)
    nc.insert_act_table_loads = types.MethodType(_patched_insert_act_table_loads, nc)

    def din(name, shape, dt=F32):
        return nc.dram_tensor(name, list(shape), dt, kind="ExternalInput").ap()

    # weights arrive pre-rearranged to the on-chip [p, chunk, n] layout so
    # every load is a contiguous [128, N] DMA (cheap descriptor issue)
    d = {"zero_bias": zero_bias}
    d["xT"] = din("xT", (P, DK, S), BF)
    d["w_in"] = din("w_in", (P, DK, D), BF)
    d["wq"] = din("wq", (P, DK, D), BF)
    d["wk"] = din("wk", (P, DK, D), BF)
    d["wv"] = din("wv", (P, DK, D), BF)
    d["wo"] = din("wo", (P, DK, D), BF)
    d["w1"] = din("w1", (P, DK, FFN), BF)
    d["w2"] = din("w2", (P, FK, D), BF)
    d["w_out"] = din("w_out", (P, DK, D), BF)
    d["qaug"] = din("qaug", (3 * H, S), BF)  # per head: ones, ones, qrow
    d["kaug"] = din("kaug", (3 * H, S), BF)  # per head: krow_hi, krow_lo, ones
    d["maskf"] = din("maskf", (P, P), BF)
    d["maskb"] = din("maskb", (P, P), BF)
    d["ident"] = din("ident", (P, P), BF)
    if not zero_bias:
        d["b_in"] = din("b_in", (D,))
        d["bo"] = din("bo", (D,))
        d["b2"] = din("b2", (D,))
        d["b_out"] = din("b_out", (D,))
        d["bv"] = din("bv", (D,))
        d["bqp"] = din("bqp", (P, DK))  # paired q bias: [2-head dims, pair]
        d["b1c"] = din("b1c", (P, FK))
    d["out"] = nc.dram_tensor("out", [S, D], F32, kind="ExternalOutput").ap()

    with tile.TileContext(nc, pool_alloc_mode="queue") as tc:
        _emit(nc, tc, d, gelu_mode)
    nc.compile()
    return nc


def _emit(nc, tc, d, gelu_mode):
    pool = tc.alloc_tile_pool
    zb = d["zero_bias"]

    pc = pool(name="consts", bufs=1)
    pw = pool(name="weights", bufs=1)
    ph = pool(name="resid", bufs=2)  # h1/h2/h3 rotate, fp32
    phT = pool(name="transposed", bufs=2)  # hn1T/attnT2/hn2T/hn3T
    phn = pool(name="hn_nat", bufs=4)
    psm = pool(name="smalls", bufs=2)
    pva = pool(name="vaug", bufs=1)
    pattn = pool(name="attn_nat", bufs=1)
    posb = pool(name="outsb", bufs=2)
    # attention-only pools: allocated last (top of the pool stack) so they
    # can be released before the FFN gt pool is allocated
    pqk = pool(name="qkheads", bufs=4)
    pqp = pool(name="qkscratch", bufs=3)
    pexp = pool(name="expT", bufs=2)

    # psum: 2x mm (1 bank) + 2x sc (2 banks) + 2x aux (1 bank) = 8 banks
    pps = pool(name="ps", bufs=2, space="PSUM")

    def ps_mm(name, shape=None):
        return pps.tile(shape or [P, 512], F32, tag="mm", name=name)

    def ps_sc(name, shape=None):
        return pps.tile(shape or [P, 1024], F32, tag="sc", name=name)

    # ---- DMAs: ident first (warmup dep), then startup-critical tensors ----
    identB = pc.tile([P, P], BF, tag="ident")
    nc.sync.dma_start(out=identB, in_=d["ident"])
    win_sb = pw.tile([P, DK, D], BF, tag="w_in")
    nc.sync.dma_start(out=win_sb, in_=d["w_in"])
    xT_sb = pw.tile([P, DK, S], BF, tag="xT")
    nc.sync.dma_start(out=xT_sb[:, :, 0:512], in_=d["xT"][:, :, 0:512])
    nc.sync.dma_start(out=xT_sb[:, :, 512:1024], in_=d["xT"][:, :, 512:1024])

    def wload(name, shape, eng=nc.sync):
        t = pw.tile(shape, BF, tag=name)
        eng.dma_start(out=t, in_=d[name])
        return t

    wq_sb = wload("wq", [P, DK, D], nc.scalar)
    wk_sb = wload("wk", [P, DK, D], nc.scalar)
    wv_sb = wload("wv", [P, DK, D], nc.scalar)

    maskf = pc.tile([P, P], BF, tag="maskf")
    nc.gpsimd.dma_start(out=maskf, in_=d["maskf"])
    maskb = pc.tile([P, P], BF, tag="maskb")
    nc.gpsimd.dma_start(out=maskb, in_=d["maskb"])

    wo_sb = wload("wo", [P, DK, D], nc.scalar)
    w1_sb = wload("w1", [P, DK, FFN], nc.sync)
    w2_sb = wload("w2", [P, FK, D], nc.gpsimd)
    wout_sb = wload("w_out", [P, DK, D], nc.sync)

    epsc = pc.tile([P, 1], F32, tag="epsc")
    nc.any.memset(epsc, EPS)

    if not zb:
        def bcast(name, shape=None):
            t = pc.tile(shape or [P, D], F32, tag=name + "B")
            nc.gpsimd.dma_start(out=t, in_=d[name].partition_broadcast(P))
            return t

        binB = bcast("b_in")
        boB = bcast("bo")
        b2B = bcast("b2")
        boutB = bcast("b_out")
        bvB = bcast("bv", [P, H, HD])
        bqp = pc.tile([P, DK], F32, tag="bqp")
        nc.gpsimd.dma_start(out=bqp, in_=d["bqp"])
        b1c = pc.tile([P, FK], F32, tag="b1c")
        nc.gpsimd.dma_start(out=b1c, in_=d["b1c"])
        b1cs = pc.tile([P, FK], F32, tag="b1cs")
        nc.any.tensor_scalar(b1cs, b1c, scalar1=1.702, scalar2=None, op0=OP.mult)

    # ---- HAM warm-up: real matmuls (transposes don't count as PE-busy) ----
    def warm(n):
        for i in range(n):
            wt = ps_mm(f"warm{i}")
            nc.tensor.matmul(wt[:, 0:128], identB, identB, start=True, stop=True)

    warm(150)

    # ---- h1 = x @ w_in (+ b_in) ----
    h1 = ph.tile([P, SM, D], F32, tag="h", name="h1")

    def emit_h1(m):
        ps = ps_mm(f"h1ps{m}")
        for dk in range(DK):
            nc.tensor.matmul(
                ps,
                xT_sb[:, dk, m * P : (m + 1) * P],
                win_sb[:, dk, :],
                start=(dk == 0),
                stop=(dk == DK - 1),
            )
        if zb:
            nc.vector.tensor_copy(h1[:, m, :], ps)
        else:
            nc.vector.tensor_tensor(out=h1[:, m, :], in0=ps, in1=binB, op=OP.add)

    # ---- batched LN: stats (DVE) + rs = exp(-0.5*ln(var+eps)) (ACT) ----
    def ln_stats(src_rows, mv, lo):
        # bn_stats/aggr for 4 rows into mv[:, lo:lo+4, :]
        for i, src in enumerate(src_rows):
            st = psm.tile([P, 6], F32, tag="st", name=f"st{lo + i}")
            nc.vector.bn_stats(st, src)
            nc.vector.bn_aggr(mv[:, lo + i, :], st)

    def ln_apply(src_rows, mv, lo, hn_tag, after=None):
        # (ln_apply.last_exp holds the most recent Exp inst for epoch pinning)
        n = len(src_rows)
        lnv = psm.tile([P, 4], F32, tag="lnv", name=f"lnv{lo}")
        ln_inst = nc.scalar.activation(
            lnv[:, 0:n], mv[:, lo : lo + n, 1], AF.Ln, bias=epsc
        )
        if after is not None:
            # keep the ACT queue in one table-set epoch: this Ln must not be
            # scheduled between Gelu ops (each crossing costs a ~2.7us
            # ACT_TABLE_LOAD)
            tile.add_dep_helper(ln_inst.ins, after.ins, reason="ln after gelu epoch")
        rs = psm.tile([P, 4], F32, tag="rs", name=f"rs{lo}")
        ln_apply.last_exp = nc.scalar.activation(
            rs[:, 0:n], lnv[:, 0:n], AF.Exp, scale=-0.5
        )
        ng = psm.tile([P, 4], F32, tag="ng", name=f"ng{lo}")
        nc.vector.tensor_tensor(
            out=ng[:, 0:n], in0=mv[:, lo : lo + n, 0], in1=rs[:, 0:n], op=OP.mult
        )
        hns = []
        for i, src in enumerate(src_rows):
            hn = phn.tile([P, D], BF, tag="hn", name=f"{hn_tag}{lo + i}")
            nc.vector.tensor_scalar(
                hn, src, scalar1=rs[:, i : i + 1], scalar2=ng[:, i : i + 1],
                op0=OP.mult, op1=OP.subtract,
            )
            hns.append(hn)
        return hns

    def transpose_row(hT, m, src):
        t4 = pps.tile([P, DK, P], BF, tag="aux", name=f"tr{m}")
        for dk in range(DK):
            nc.tensor.transpose(t4[:, dk, :], src[:, dk * P : (dk + 1) * P], identB)
        nc.vector.tensor_copy(hT[:, :, m * P : (m + 1) * P], t4)

    # ---- v projection into v_aug [t, m, h, hd+denom] ----
    hn1T = phT.tile([P, DK, S], BF, tag="hT", name="hn1T")
    v_aug = pva.tile([P, SM, H, HD + 1], BF, tag="vaug")
    nc.gpsimd.memset(v_aug[:, :, :, HD : HD + 1], 1.0)

    def emit_v(m):
        psv = ps_mm(f"psv{m}", [P, H, HD])
        for dk in range(DK):
            nc.tensor.matmul(
                psv,
                hn1T[:, dk, m * P : (m + 1) * P],
                wv_sb[:, dk, :],
                start=(dk == 0),
                stop=(dk == DK - 1),
            )
        if zb:
            nc.vector.tensor_copy(v_aug[:, m, :, 0:HD], psv)
        else:
            nc.vector.tensor_tensor(out=v_aug[:, m, :, 0:HD], in0=psv, in1=bvB, op=OP.add)

    mv1 = psm.tile([P, SM, 2], F32, tag="mv", name="mv1")
    for m in range(4):
        emit_h1(m)
    ln_stats([h1[:, m, :] for m in range(4)], mv1, 0)
    for m in range(4, SM):
        emit_h1(m)
    hnA = ln_apply([h1[:, m, :] for m in range(4)], mv1, 0, "hn1_")
    ln_stats([h1[:, m, :] for m in range(4, SM)], mv1, 4)
    for m in range(4):
        transpose_row(hn1T, m, hnA[m])
        emit_v(m)
    hnB = ln_apply([h1[:, m, :] for m in range(4, SM)], mv1, 4, "hn1_")
    for m in range(4, SM):
        transpose_row(hn1T, m, hnB[m - 4])
        emit_v(m)

    # ---- attention ----
    attn_nat = pattn.tile([P, SM, D], BF, tag="attn")
    qk_t = {}
    exp_t = {}

    def emit_qk(p):
        ha, hb = 2 * p, 2 * p + 1
        tiles = {}
        for h in (ha, hb):
            qTa = pqk.tile([HD + AUG, S], BF, tag="qTa", name=f"qTa{h}")
            kTa = pqk.tile([HD + AUG, S], BF, tag="kTa", name=f"kTa{h}")
            nc.gpsimd.dma_start(
                out=qTa[HD : HD + AUG, :], in_=d["qaug"][3 * h : 3 * h + AUG, :]
            )
            nc.gpsimd.dma_start(
                out=kTa[HD : HD + AUG, :], in_=d["kaug"][3 * h : 3 * h + AUG, :]
            )
            tiles[h] = (qTa, kTa)
        for w_sb, is_q in ((wq_sb, True), (wk_sb, False)):
            qp = pqp.tile([P, S], BF, tag="qp", name=f"qp{p}{int(is_q)}")
            for half in range(2):
                psq = ps_mm(f"psq{p}{int(is_q)}{half}")
                for dk in range(DK):
                    nc.tensor.matmul(
                        psq,
                        w_sb[:, dk, p * P : (p + 1) * P],
                        hn1T[:, dk, half * 512 : (half + 1) * 512],
                        start=(dk == 0),
                        stop=(dk == DK - 1),
                    )
                dst = qp[:, half * 512 : (half + 1) * 512]
                if is_q and not zb:
                    nc.vector.tensor_scalar(
                        dst, psq, scalar1=bqp[:, p : p + 1], scalar2=None, op0=OP.add
                    )
                else:
                    nc.vector.tensor_copy(dst, psq)
            # partition-shift the two heads' halves into the aug tiles
            dst_a = tiles[ha][0] if is_q else tiles[ha][1]
            dst_b = tiles[hb][0] if is_q else tiles[hb][1]
            nc.sync.dma_start(out=dst_a[0:HD, :], in_=qp[0:HD, :])
            nc.sync.dma_start(out=dst_b[0:HD, :], in_=qp[HD:P, :])
        qk_t[ha] = tiles[ha]
        qk_t[hb] = tiles[hb]

    def emit_scores_tile(h, ti):
        qTa, kTa = qk_t[h]
        expT = exp_t[h]
        t0, tw, tp = _score_tiles(h)[ti]
        sc = ps_sc(f"sc{h}_{ti}")
        for (j, ss, do, w) in tp:
            nc.tensor.matmul(
                sc[:, do : do + w],
                kTa[:, j * P : (j + 1) * P],
                qTa[:, ss : ss + w],
                start=True,
                stop=True,
            )
        nc.scalar.activation(expT[:, t0 : t0 + tw], sc[:, 0:tw], AF.Exp, scale=0.125)
        # diagonal masks living in this tile
        msk = maskf if _fwd(h) else maskb
        for j in range(SM):
            dg = _diag_off(h, j)
            if t0 <= dg < t0 + tw:
                nc.vector.tensor_tensor(
                    out=expT[:, dg : dg + P], in0=expT[:, dg : dg + P], in1=msk,
                    op=OP.mult,
                )

    def alloc_expT(h):
        small = _band(h) <= 2
        tag = "expS" if small else "expL"
        width = EW_S if small else EW_L
        exp_t[h] = pexp.tile([P, width], BF, tag=tag, name=f"expT{h}")

    def emit_pv_group(h, mg):
        expT = exp_t[h]
        pvt = pps.tile([P, 4, HD + 1], F32, tag="aux", name=f"pv{h}_{mg}")
        for mi in range(4):
            m = mg * 4 + mi
            js = _pv_js(h, m)
            for i, j in enumerate(js):
                col = _pv_col(h, j, m)
                nc.tensor.matmul(
                    pvt[:, mi, :],
                    expT[:, col : col + P],
                    v_aug[:, j, h, :],
                    start=(i == 0),
                    stop=(i == len(js) - 1),
                )
        rinv = psm.tile([P, 4], F32, tag="rinv", name=f"rinv{h}_{mg}")
        nc.vector.reciprocal(rinv, pvt[:, :, HD])
        for mi in range(4):
            m = mg * 4 + mi
            nc.vector.tensor_scalar(
                attn_nat[:, m, h * HD : (h + 1) * HD],
                pvt[:, mi, 0:HD],
                scalar1=rinv[:, mi : mi + 1],
                scalar2=None,
                op0=OP.mult,
            )

    # software pipeline over head pairs: qk(p) | scores(p-1) | pv(p-2)
    for step in range(6):
        if step < 4:
            emit_qk(step)
        work = []
        if 1 <= step <= 4:
            ha, hb = 2 * (step - 1), 2 * (step - 1) + 1
            alloc_expT(ha)
            alloc_expT(hb)
            for h in (ha, hb):
                for ti in range(len(_score_tiles(h))):
                    work.append(("sc", h, ti))
        if step >= 2:
            for h in (2 * (step - 2), 2 * (step - 2) + 1):
                for mg in range(2):
                    work.append(("pv", h, mg))
        # interleave score-tile fills with pv groups so PE never parks
        # behind the ACT exp drain of the sc psum rotation
        scw = [w for w in work if w[0] == "sc"]
        pvw = [w for w in work if w[0] == "pv"]
        out = []
        si, pi = 0, 0
        while si < len(scw) or pi < len(pvw):
            if si < len(scw):
                out.append(scw[si]); si += 1
            if pi < len(pvw):
                out.append(pvw[pi]); pi += 1
        for kind, h, idx in out:
            if kind == "sc":
                emit_scores_tile(h, idx)
            else:
                emit_pv_group(h, idx)
        for h in (2 * (step - 2), 2 * (step - 2) + 1) if step >= 2 else ():
            exp_t.pop(h)
            qk_t.pop(h)

    # ---- attnT2 + wo + h2 ----
    attnT2 = phT.tile([P, DK, S], BF, tag="hT", name="attnT2")
    h2 = ph.tile([P, SM, D], F32, tag="h", name="h2")

    def emit_wo_h2(m):
        transpose_row(attnT2, m, attn_nat[:, m, :])
        ps = ps_mm(f"pswo{m}")
        for dk in range(DK):
            nc.tensor.matmul(
                ps,
                attnT2[:, dk, m * P : (m + 1) * P],
                wo_sb[:, dk, :],
                start=(dk == 0),
                stop=(dk == DK - 1),
            )
        nc.vector.tensor_tensor(out=h2[:, m, :], in0=ps, in1=h1[:, m, :], op=OP.add)
        if not zb:
            nc.gpsimd.tensor_tensor(out=h2[:, m, :], in0=h2[:, m, :], in1=boB, op=OP.add)

    mv2 = psm.tile([P, SM, 2], F32, tag="mv", name="mv2")
    hn2T = phT.tile([P, DK, S], BF, tag="hT", name="hn2T")
    for m in range(4):
        emit_wo_h2(m)
    ln_stats([h2[:, m, :] for m in range(4)], mv2, 0)
    for m in range(4, SM):
        emit_wo_h2(m)
    hn2A = ln_apply([h2[:, m, :] for m in range(4)], mv2, 0, "hn2_")
    ln_stats([h2[:, m, :] for m in range(4, SM)], mv2, 4)
    for m in range(4):
        transpose_row(hn2T, m, hn2A[m])
    hn2B = ln_apply([h2[:, m, :] for m in range(4, SM)], mv2, 4, "hn2_")
    for m in range(4, SM):
        transpose_row(hn2T, m, hn2B[m - 4])

    # release attention-phase SBUF (LIFO) before allocating the FFN gt stage
    pexp.release()
    pqp.release()
    pqk.release()

    pg = tc.alloc_tile_pool(name="gelu", bufs=2)

    # ---- FFN: per half, ffn1+gelu into gt, then ffn2 per s-chunk ----
    h3 = ph.tile([P, SM, D], F32, tag="h", name="h3")

    def emit_ffn1_group(half, g, gt):
        # kc pair (2g, 2g+1) -> one 2-bank psum tile -> one gelu
        sc = ps_sc(f"f1_{half}_{g}", [P, 2, 512])
        for i in range(2):
            kc = 2 * g + i
            for dk in range(DK):
                nc.tensor.matmul(
                    sc[:, i, :],
                    w1_sb[:, dk, kc * P : (kc + 1) * P],
                    hn2T[:, dk, half * 512 : (half + 1) * 512],
                    start=(dk == 0),
                    stop=(dk == DK - 1),
                )
        if gelu_mode == "gelu":
            if zb:
                return nc.scalar.activation(gt[:, 2 * g : 2 * g + 2, :], sc, AF.Gelu)
            last = None
            for i in range(2):
                kc = 2 * g + i
                last = nc.scalar.activation(
                    gt[:, kc, :], sc[:, i, :], AF.Gelu, bias=b1c[:, kc : kc + 1]
                )
            return last
        # CoreSim lacks Gelu: x*sigmoid(1.702x) stand-in
        last = None
        for i in range(2):
            kc = 2 * g + i
            scs = sc[:, i, :]
            sg = pg.tile([P, 512], F32, tag="sg")
            if zb:
                last = nc.scalar.activation(sg, scs, AF.Sigmoid, scale=1.702)
                nc.vector.tensor_tensor(out=gt[:, kc, :], in0=sg, in1=scs, op=OP.mult)
            else:
                last = nc.scalar.activation(
                    sg, scs, AF.Sigmoid, bias=b1cs[:, kc : kc + 1], scale=1.702
                )
                xb = pg.tile([P, 512], F32, tag="xb")
                nc.any.tensor_scalar(
                    xb, scs, scalar1=b1c[:, kc : kc + 1], scalar2=None, op0=OP.add
                )
                nc.any.tensor_tensor(out=gt[:, kc, :], in0=sg, in1=xb, op=OP.mult)
        return last

    def emit_ffn2_m(half, mm, gt):
        acc = ps_mm(f"f2_{half}_{mm}")
        for kc in range(FK):
            nc.tensor.matmul(
                acc,
                gt[:, kc, mm * P : (mm + 1) * P],
                w2_sb[:, kc, :],
                start=(kc == 0),
                stop=(kc == FK - 1),
            )
        m = half * 4 + mm
        nc.vector.tensor_tensor(out=h3[:, m, :], in0=acc, in1=h2[:, m, :], op=OP.add)
        if not zb:
            nc.gpsimd.tensor_tensor(out=h3[:, m, :], in0=h3[:, m, :], in1=b2B, op=OP.add)

    gts = {}
    for half in range(2):
        gts[half] = pg.tile([P, FK, 512], BF, tag="gt", name=f"gt{half}")
    # chain the gelu ops after LN2-B's Exp and after each other so the
    # scheduler cannot interleave them with ln/exp ops (ACT table thrash)
    last_gelu = ln_apply.last_exp
    for g in range(8):
        prev, last_gelu = last_gelu, emit_ffn1_group(0, g, gts[0])
        tile.add_dep_helper(last_gelu.ins, prev.ins, reason="gelu epoch chain")
    # interleave ffn2(half 0) with ffn1(half 1)
    for i in range(8):
        if i < 4:
            emit_ffn2_m(0, i, gts[0])
        prev, last_gelu = last_gelu, emit_ffn1_group(1, i, gts[1])
        tile.add_dep_helper(last_gelu.ins, prev.ins, reason="gelu epoch chain")
    mvf = psm.tile([P, SM, 2], F32, tag="mv", name="mvf")
    ln_stats([h3[:, m, :] for m in range(4)], mvf, 0)
    for mm in range(4):
        emit_ffn2_m(1, mm, gts[1])

    # ---- LNf + w_out tail ----
    hn3T = phT.tile([P, DK, S], BF, tag="hT", name="hn3T")
    out_view = d["out"].rearrange("(c p) n -> p c n", p=P)

    def emit_wout(m):
        ps = ps_mm(f"psout{m}")
        for dk in range(DK):
            nc.tensor.matmul(
                ps,
                hn3T[:, dk, m * P : (m + 1) * P],
                wout_sb[:, dk, :],
                start=(dk == 0),
                stop=(dk == DK - 1),
            )
        osb = posb.tile([P, D], F32, tag="osb")
        if zb:
            nc.vector.tensor_copy(osb, ps)
        else:
            nc.vector.tensor_tensor(out=osb, in0=ps, in1=boutB, op=OP.add)
        nc.sync.dma_start(out=out_view[:, m, :], in_=osb)

    hnfA = ln_apply([h3[:, m, :] for m in range(4)], mvf, 0, "hn3_", after=last_gelu)
    ln_stats([h3[:, m, :] for m in range(4, SM)], mvf, 4)
    for m in range(4):
        transpose_row(hn3T, m, hnfA[m])
        emit_wout(m)
    hnfB = ln_apply([h3[:, m, :] for m in range(4, SM)], mvf, 4, "hn3_")
    for m in range(4, SM):
        transpose_row(hn3T, m, hnfB[m - 4])
        emit_wout(m)

    for p_ in (pg, posb, pattn, pva, psm, phn, phT, ph, pw, pc, pps):
        p_.release()


def host_prep(inputs):
    """Fold LN affine params into weights; build ALiBi helper tensors."""
    f = lambda k: np.asarray(inputs[k], dtype=np.float64)
    ln1_s, ln1_b = f("ln1_s"), f("ln1_b")
    ln2_s, ln2_b = f("ln2_s"), f("ln2_b")
    lnf_s, lnf_b = f("lnf_s"), f("lnf_b")
    wq, bq = f("wq"), f("bq")
    wk = f("wk")
    wv, bv = f("wv"), f("bv")
    w1, b1 = f("w1"), f("b1")
    w_out, b_out = f("w_out"), f("b_out")

    wq_f = ln1_s[:, None] * wq
    bq_f = (bq + ln1_b @ wq).astype(np.float32)
    wk_f = ln1_s[:, None] * wk
    wv_f = ln1_s[:, None] * wv
    bv_f = (bv + ln1_b @ wv).astype(np.float32)
    w1_f = ln2_s[:, None] * w1
    b1_f = (b1 + ln2_b @ w1).astype(np.float32)
    wout_f = lnf_s[:, None] * w_out
    bout_f = (b_out + lnf_b @ w_out).astype(np.float32)

    sl = _slopes()
    qaug = np.zeros((H, 3, S), np.float64)
    kaug = np.zeros((H, 3, S), np.float64)
    s_idx = np.arange(S, dtype=np.float64)
    for h in range(H):
        sgn = -1.0 if h < H // 2 else 1.0  # sign of the per-s row term
        kraw = -sgn * 8.0 * sl[h % 4] * s_idx  # per-t term, rides kTa aug rows
        hi = kraw.astype(BF_NP).astype(np.float64)
        kaug[h, 0] = hi
        kaug[h, 1] = kraw - hi  # bf16 residual: per-t term exact to ~2^-18
        kaug[h, 2] = 1.0
        qaug[h, 0] = 1.0
        qaug[h, 1] = 1.0
        qaug[h, 2] = sgn * 8.0 * sl[h % 4] * s_idx  # per-s term (cancels)
    qaug = qaug.reshape(3 * H, S)
    kaug = kaug.reshape(3 * H, S)
    maskf = np.triu(np.ones((P, P), np.float32))  # keep t <= s (p <= c)
    maskb = np.tril(np.ones((P, P), np.float32))  # keep t >= s (p >= c)

    bf = lambda a: np.ascontiguousarray(np.asarray(a, np.float32).astype(BF_NP))

    def chunked(w):
        # [K, N] -> on-chip [p, c, n] layout with K = c*128 + p
        w = np.asarray(w, np.float32)
        k, n = w.shape
        return bf(w.reshape(k // P, P, n).transpose(1, 0, 2))

    zero_bias = all(
        float(np.abs(a).max()) == 0.0
        for a in (f("b_in"), bq_f, bv_f, f("bo"), b1_f, f("b2"), bout_f)
    )
    common = {
        "w_in": chunked(inputs["w_in"]),
        "wq": chunked(wq_f),
        "wk": chunked(wk_f),
        "wv": chunked(wv_f),
        "wo": chunked(inputs["wo"]),
        "w1": chunked(w1_f),
        "w2": chunked(inputs["w2"]),
        "w_out": chunked(wout_f),
        "qaug": bf(qaug),
        "kaug": bf(kaug),
        "maskf": bf(maskf),
        "maskb": bf(maskb),
        "ident": bf(np.eye(P, dtype=np.float32)),
    }
    if not zero_bias:
        common.update(
            {
                "b_in": np.asarray(inputs["b_in"], np.float32),
                "bo": np.asarray(inputs["bo"], np.float32),
                "b2": np.asarray(inputs["b2"], np.float32),
                "b_out": bout_f,
                "bv": bv_f,
                # paired q bias: [128 dims of the pair, pair index]
                "bqp": np.ascontiguousarray(bq_f.reshape(DK, P).T),
                "b1c": np.ascontiguousarray(b1_f.reshape(FK, P).T),
            }
        )
    return common, zero_bias


def core_map(common, x, i):
    xT = np.asarray(x[i], np.float32).T  # [D, S]
    xT = np.ascontiguousarray(
        xT.reshape(DK, P, S).transpose(1, 0, 2).astype(BF_NP)
    )
    return dict(common, xT=xT)


_NC_CACHE = {}


def get_nc(gelu_mode="gelu", zero_bias=True):
    key = (gelu_mode, zero_bias)
    if key not in _NC_CACHE:
        _NC_CACHE[key] = build_nc(gelu_mode, zero_bias)
    return _NC_CACHE[key]


def run(inputs, trace=False, tmpdir=None):
    common, zero_bias = host_prep(inputs)
    x = np.asarray(inputs["x"], np.float32)
    in_maps = [core_map(common, x, i) for i in range(N_CORES)]
    nc = get_nc("gelu", zero_bias)
    res = run_bass_kernel_spmd(
        nc, in_maps, core_ids=list(range(N_CORES)), trace=trace, tmpdir=tmpdir
    )
    out = np.stack([res.results[i]["out"] for i in range(N_CORES)])
    return out.astype(np.float32), res


def kernel(**inputs):
    out, _ = run(inputs, trace=False)
    return out
